# revision 7
# baseline (speedup 1.0000x reference)
"""LSTM encoder kernel for TRN2 (8 NeuronCores, data-parallel over batch).

Reference computation:
  x = feats @ W_embed.T + b_embed            [B,T,F] -> [B,T,E]
  per t: gates = x_t @ W_ih.T + b_ih + h @ W_hh.T + b_hh
         i,f,g,o = split(gates); c = sig(f)*c + sig(i)*tanh(g); h = sig(o)*tanh(c)
  out = stacked h                            [B,T,H]

B=256, T=64, F=2048, E=512, H=512.  Shard B over 8 cores (B_L=32 each).

Device-side design (per core):
  Phase 1: xT[E, T*B_L] = W_emb @ featsT in float32r (feats pre-transposed on
           host so the per-step slice xT[:, t*32:(t+1)*32] is directly the
           lhsT of step t).
  Phase 2: per step, col-tiled fp16 matmuls (tile_position=(0,32j)) produce
           gates in PSUM with layout [(j=h-chunk, b) partitions, (gate,h')
           free] so all elementwise LSTM math runs on full 128 partitions.
           Gate columns are pre-permuted on host to (i,f,o,g) so one sigmoid
           covers free[0:384] and one tanh covers free[384:512].
           h is re-transposed for the next step with 4 row-tiled PE
           transposes.
  Matmuls: embedding fp32r (1 cyc/row at N>=256), recurrence fp16 (1 cyc/row,
  col-tiling legal, ~1.4e-3 rel err end to end).  Elementwise fp32.
"""

import numpy as np

B, T, F, E, H = 256, 64, 2048, 512, 512
NC = 8
BL = B // NC          # 32 batch rows per core
G4 = 4 * H            # 2048 gate columns
NJ = 4                # h-chunks of 128 (and batch col-groups of 32)
HP = H // NJ          # 128

_prog_cache = {}


def _build_program(has_bias: bool, bench_loop: int = 0):
    import concourse.bass as bass
    import concourse.tile as tile
    from concourse import bacc, mybir
    from concourse.masks import make_identity
    from contextlib import ExitStack

    f32 = mybir.dt.float32
    f32r = mybir.dt.float32r
    f16 = mybir.dt.float16

    nc = bacc.Bacc("TRN2", target_bir_lowering=False, debug=False)

    featsT = nc.dram_tensor("featsT", [F, T * BL], f32r, kind="ExternalInput").ap()
    wembT = nc.dram_tensor("wembT", [F, E], f32r, kind="ExternalInput").ap()
    wrec = nc.dram_tensor("wrec", [2 * E, G4], f16, kind="ExternalInput").ap()
    if has_bias:
        biasg = nc.dram_tensor("biasg", [128, 512], f32, kind="ExternalInput").ap()
    out = nc.dram_tensor("out", [T, 128, HP], f32, kind="ExternalOutput").ap()

    with tile.TileContext(nc) as tc:
        with ExitStack() as ctx:
            const_pool = ctx.enter_context(tc.tile_pool(name="const", bufs=1))
            state_pool = ctx.enter_context(tc.tile_pool(name="state", bufs=1))
            embin_pool = ctx.enter_context(tc.tile_pool(name="emb_in", bufs=2))
            embps_pool = ctx.enter_context(
                tc.tile_pool(name="emb_ps", bufs=2, space="PSUM"))
            pg_pool = ctx.enter_context(
                tc.tile_pool(name="pg", bufs=2, space="PSUM"))
            pt_pool = ctx.enter_context(
                tc.tile_pool(name="pt", bufs=4, space="PSUM"))
            ew_pool = ctx.enter_context(tc.tile_pool(name="ew", bufs=3))

            # I32 replicated in each 32-partition block, so the transpose
            # matmul's rhs can sit at the same base partition as its lhsT.
            ident = const_pool.tile([128, 32], f32)
            for j in range(NJ):
                make_identity(nc, ident[32 * j:32 * (j + 1), :])

            xT = const_pool.tile([128, NJ, T * BL], f16)         # 2 MB
            wrec_sb = const_pool.tile([128, 8, G4], f16)         # 4 MB
            nc.sync.dma_start(
                wrec_sb[:], wrec.rearrange("(ko p) n -> p ko n", p=128)
            )
            wemb_sb = const_pool.tile([128, 16, E], f32r)        # 4 MB
            nc.sync.dma_start(
                wemb_sb[:], wembT.rearrange("(ko p) m -> p ko m", p=128)
            )
            if has_bias:
                bias_sb = const_pool.tile([128, 512], f32)
                nc.sync.dma_start(bias_sb[:], biasg[:])

            hT = state_pool.tile([128, NJ, BL], f16)             # h transposed
            c_sb = state_pool.tile([128, HP], f32)               # cell state

            def body():
                nc.vector.memzero(hT[:])
                nc.vector.memzero(c_sb[:])

                # ---- phase 1: xT = W_emb @ featsT (fp32r) ----
                NCH = 256
                NCHUNKS = (T * BL) // NCH
                featsT_r = featsT.rearrange("(ko p) n -> p ko n", p=128)
                for nch in range(NCHUNKS):
                    rhs = embin_pool.tile([128, 16, NCH], f32r, tag="embrhs")
                    nc.sync.dma_start(
                        rhs[:], featsT_r[:, :, nch * NCH:(nch + 1) * NCH]
                    )
                    for m in range(NJ):
                        ps = embps_pool.tile([128, NCH], f32, tag="embps")
                        for ko in range(16):
                            nc.tensor.matmul(
                                ps[:],
                                wemb_sb[:, ko, m * 128:(m + 1) * 128],
                                rhs[:, ko, :],
                                start=(ko == 0),
                                stop=(ko == 15),
                            )
                        nc.vector.tensor_copy(
                            xT[:, m, nch * NCH:(nch + 1) * NCH], ps[:]
                        )

                # ---- phase 2: recurrence (fp16 col-tiled matmuls) ----
                for t in range(T):
                    psg = pg_pool.tile([128, 512], f32, tag="psg")
                    # x-part first (independent of h(t-1): overlaps with the
                    # previous step's elementwise tail), then h-part.
                    for ko in (4, 5, 6, 7, 0, 1, 2, 3):
                        if ko >= 4:
                            lhsT = xT[:, ko - 4, t * BL:(t + 1) * BL]
                        else:
                            lhsT = hT[:, ko, :]
                        for j in range(NJ):
                            nc.tensor.matmul(
                                psg[32 * j:32 * (j + 1), :],
                                lhsT,
                                wrec_sb[:, ko, j * 512:(j + 1) * 512],
                                start=(ko == 4),
                                stop=(ko == 3),
                                tile_position=(0, 32 * j),
                            )
                    if has_bias:
                        nc.vector.tensor_add(psg[:], psg[:], bias_sb[:])
                    acts = ew_pool.tile([128, 512], f32, tag="acts")
                    nc.scalar.activation(
                        acts[:, 0:384], psg[:, 0:384],
                        mybir.ActivationFunctionType.Sigmoid,
                    )
                    nc.scalar.activation(
                        acts[:, 384:512], psg[:, 384:512],
                        mybir.ActivationFunctionType.Tanh,
                    )
                    # free slices: i [0:128], f [128:256], o [256:384], g [384:512]
                    fc = ew_pool.tile([128, HP], f32, tag="fc")
                    nc.vector.tensor_mul(fc[:], acts[:, 128:256], c_sb[:])
                    ig = ew_pool.tile([128, HP], f32, tag="ig")
                    nc.vector.tensor_mul(ig[:], acts[:, 0:128], acts[:, 384:512])
                    nc.vector.tensor_add(c_sb[:], fc[:], ig[:])
                    tanh_c = ew_pool.tile([128, HP], f32, tag="tanh_c")
                    nc.scalar.activation(
                        tanh_c[:], c_sb[:], mybir.ActivationFunctionType.Tanh
                    )
                    h_new = ew_pool.tile([128, HP], f32, tag="h_new")
                    nc.vector.tensor_mul(h_new[:], acts[:, 256:384], tanh_c[:])
                    nc.sync.dma_start(out[t], h_new[:])
                    if t + 1 < T:
                        for j in range(NJ):
                            tp = pt_pool.tile([128, BL], f32, tag="tp")
                            nc.tensor.matmul(
                                tp[:],
                                h_new[32 * j:32 * (j + 1), :],
                                ident[32 * j:32 * (j + 1), :],
                                is_transpose=True,
                                tile_position=(32 * j, 0),
                            )
                            nc.vector.tensor_copy(hT[:, j, :], tp[:])

            if bench_loop:
                with tc.For_i(0, bench_loop, 1):
                    body()
            else:
                body()

    nc.compile()
    return nc


def _prep_inputs(feats_videos, W_embed, b_embed, W_ih, W_hh, b_ih, b_hh):
    """Host-side shard + relayout. Returns (in_maps, has_bias)."""
    f32 = np.float32
    # Combined recurrence weights: rows 0:511 = W_hh.T (h part),
    # rows 512:1023 = W_ih.T (x part).  Columns reordered to
    # col = jchunk*512 + gatepos*128 + h', gate order (i,f,o,g).
    W_cat = np.concatenate([W_hh.T, W_ih.T], axis=0).astype(f32)  # [1024, 2048]
    arr = W_cat.reshape(2 * E, 4, NJ, HP)       # [k, gate_orig, jchunk, h']
    arr = arr[:, [0, 1, 3, 2], :, :]            # gate order -> (i, f, o, g)
    wrec_np = np.ascontiguousarray(
        arr.transpose(0, 2, 1, 3).reshape(2 * E, G4)
    ).astype(np.float16)

    wembT_np = np.ascontiguousarray(W_embed.T.astype(f32))  # [F, E]

    # total gate bias, in the same [(j,b), (gatepos,h')] layout as psum
    b_g = (W_ih @ b_embed + b_ih + b_hh).astype(f32)        # [2048]
    has_bias = bool(np.any(b_g))
    bias_np = None
    if has_bias:
        bg = b_g.reshape(4, NJ, HP)[[0, 1, 3, 2], :, :]     # [gatepos, j, h']
        bias_tile = np.empty((128, 512), dtype=f32)
        for j in range(NJ):
            for gp in range(4):
                bias_tile[32 * j:32 * (j + 1), 128 * gp:128 * (gp + 1)] = bg[gp, j][None, :]
        bias_np = bias_tile

    in_maps = []
    for c in range(NC):
        fl = feats_videos[c * BL:(c + 1) * BL]              # [32, 64, 2048]
        featsT_np = np.ascontiguousarray(
            fl.transpose(2, 1, 0).reshape(F, T * BL).astype(f32)
        )
        m = {"featsT": featsT_np, "wembT": wembT_np, "wrec": wrec_np}
        if has_bias:
            m["biasg"] = bias_np
        in_maps.append(m)
    return in_maps, has_bias


def kernel(feats_videos, W_embed, b_embed, W_ih, W_hh, b_ih, b_hh):
    import sys
    if "/opt/trn_rl_repo" not in sys.path:
        sys.path.insert(0, "/opt/trn_rl_repo")
    from concourse.bass_utils import run_bass_kernel_spmd

    feats_videos = np.asarray(feats_videos, dtype=np.float32)
    W_embed = np.asarray(W_embed, dtype=np.float32)
    b_embed = np.asarray(b_embed, dtype=np.float32)
    W_ih = np.asarray(W_ih, dtype=np.float32)
    W_hh = np.asarray(W_hh, dtype=np.float32)
    b_ih = np.asarray(b_ih, dtype=np.float32)
    b_hh = np.asarray(b_hh, dtype=np.float32)

    in_maps, has_bias = _prep_inputs(
        feats_videos, W_embed, b_embed, W_ih, W_hh, b_ih, b_hh
    )
    if has_bias not in _prog_cache:
        _prog_cache[has_bias] = _build_program(has_bias)
    nc = _prog_cache[has_bias]

    res = run_bass_kernel_spmd(nc, in_maps, list(range(NC)))
    outs = []
    for c in range(NC):
        r = res.results[c]["out"]                            # [64, 128, 128]
        o = r.reshape(T, NJ, BL, HP).transpose(2, 0, 1, 3).reshape(BL, T, H)
        outs.append(o)
    return np.concatenate(outs, axis=0).astype(np.float32)   # [256, 64, 512]


# revision 8
# speedup vs baseline: 1.9482x; 1.9482x over previous
"""LSTM encoder kernel for TRN2 (8 NeuronCores, data-parallel over batch).

Reference computation:
  x = feats @ W_embed.T + b_embed            [B,T,F] -> [B,T,E]
  per t: gates = x_t @ W_ih.T + b_ih + h @ W_hh.T + b_hh
         i,f,g,o = split(gates); c = sig(f)*c + sig(i)*tanh(g); h = sig(o)*tanh(c)
  out = stacked h                            [B,T,H]

B=256, T=64, F=2048, E=512, H=512.  Shard B over 8 cores (B_L=32 each).

Device-side design (per core):
  Phase 1: xT[E, T*B_L] = W_emb @ featsT in float32r (feats pre-transposed on
           host so the per-step slice xT[:, t*32:(t+1)*32] is directly the
           lhsT of step t).
  Phase 2: per step, col-tiled fp16 matmuls (tile_position=(0,32j)) produce
           gates in PSUM with layout [(j=h-chunk, b) partitions, (gate,h')
           free] so all elementwise LSTM math runs on full 128 partitions.
           Gate columns are pre-permuted on host to (i,f,o,g) so one sigmoid
           covers free[0:384] and one tanh covers free[384:512].
           h is re-transposed for the next step with 4 row-tiled PE
           transposes.
  Matmuls: embedding fp32r (1 cyc/row at N>=256), recurrence fp16 (1 cyc/row,
  col-tiling legal, ~1.4e-3 rel err end to end).  Elementwise fp32.
"""

import numpy as np

B, T, F, E, H = 256, 64, 2048, 512, 512
NC = 8
BL = B // NC          # 32 batch rows per core
G4 = 4 * H            # 2048 gate columns
NJ = 4                # h-chunks of 128 (and batch col-groups of 32)
HP = H // NJ          # 128

_prog_cache = {}


def _build_program(has_bias: bool, bench_loop: int = 0, mode: str = "all"):
    import concourse.bass as bass
    import concourse.tile as tile
    from concourse import bacc, mybir
    from concourse.masks import make_identity
    from contextlib import ExitStack

    f32 = mybir.dt.float32
    f32r = mybir.dt.float32r
    f16 = mybir.dt.float16

    nc = bacc.Bacc("TRN2", target_bir_lowering=False, debug=False)

    featsT = nc.dram_tensor("featsT", [F, T * BL], f32r, kind="ExternalInput").ap()
    wembT = nc.dram_tensor("wembT", [F, E], f32r, kind="ExternalInput").ap()
    wrec = nc.dram_tensor("wrec", [2 * E, G4], f16, kind="ExternalInput").ap()
    if has_bias:
        biasg = nc.dram_tensor("biasg", [128, 512], f32, kind="ExternalInput").ap()
    out = nc.dram_tensor("out", [T, 128, HP], f32, kind="ExternalOutput").ap()

    with tile.TileContext(nc) as tc:
        with ExitStack() as ctx:
            const_pool = ctx.enter_context(tc.tile_pool(name="const", bufs=1))
            state_pool = ctx.enter_context(tc.tile_pool(name="state", bufs=1))
            embin_pool = ctx.enter_context(tc.tile_pool(name="emb_in", bufs=2))
            embps_pool = ctx.enter_context(
                tc.tile_pool(name="emb_ps", bufs=2, space="PSUM"))
            pg_pool = ctx.enter_context(
                tc.tile_pool(name="pg", bufs=2, space="PSUM"))
            pt_pool = ctx.enter_context(
                tc.tile_pool(name="pt", bufs=4, space="PSUM"))
            ew_pool = ctx.enter_context(tc.tile_pool(name="ew", bufs=3))

            # I32 replicated in each 32-partition block, so the transpose
            # matmul's rhs can sit at the same base partition as its lhsT.
            ident = const_pool.tile([128, 32], f32)
            for j in range(NJ):
                make_identity(nc, ident[32 * j:32 * (j + 1), :])

            xT = const_pool.tile([128, NJ, T * BL], f16)         # 2 MB
            wrec_sb = const_pool.tile([128, 8, G4], f16)         # 4 MB
            nc.sync.dma_start(
                wrec_sb[:], wrec.rearrange("(ko p) n -> p ko n", p=128)
            )
            wemb_sb = const_pool.tile([128, 16, E], f32r)        # 4 MB
            nc.sync.dma_start(
                wemb_sb[:], wembT.rearrange("(ko p) m -> p ko m", p=128)
            )
            if has_bias:
                bias_sb = const_pool.tile([128, 512], f32)
                nc.sync.dma_start(bias_sb[:], biasg[:])

            hT = state_pool.tile([128, NJ, BL], f16)             # h transposed
            c_sb = state_pool.tile([128, HP], f32)               # cell state

            def body():
                nc.vector.memzero(hT[:])
                nc.vector.memzero(c_sb[:])

                if mode == "rec":
                    pass  # skip embedding (xT left stale)
                # ---- phase 1: xT = W_emb @ featsT (fp32r) ----
                NCH = 256
                NCHUNKS = 0 if mode == "rec" else (T * BL) // NCH
                featsT_r = featsT.rearrange("(ko p) n -> p ko n", p=128)
                for nch in range(NCHUNKS):
                    rhs = embin_pool.tile([128, 16, NCH], f32r, tag="embrhs")
                    nc.sync.dma_start(
                        rhs[:], featsT_r[:, :, nch * NCH:(nch + 1) * NCH]
                    )
                    for m in range(NJ):
                        ps = embps_pool.tile([128, NCH], f32, tag="embps")
                        for ko in range(16):
                            nc.tensor.matmul(
                                ps[:],
                                wemb_sb[:, ko, m * 128:(m + 1) * 128],
                                rhs[:, ko, :],
                                start=(ko == 0),
                                stop=(ko == 15),
                            )
                        nc.vector.tensor_copy(
                            xT[:, m, nch * NCH:(nch + 1) * NCH], ps[:]
                        )

                # ---- phase 2: recurrence (fp16 col-tiled matmuls) ----
                for t in range(0 if mode == "emb" else T):
                    psg = pg_pool.tile([128, 512], f32, tag="psg")
                    # x-part first (independent of h(t-1): overlaps with the
                    # previous step's elementwise tail), then h-part.
                    for ko in (4, 5, 6, 7, 0, 1, 2, 3):
                        if ko >= 4:
                            lhsT = xT[:, ko - 4, t * BL:(t + 1) * BL]
                        else:
                            lhsT = hT[:, ko, :]
                        for j in range(NJ):
                            nc.tensor.matmul(
                                psg[32 * j:32 * (j + 1), :],
                                lhsT,
                                wrec_sb[:, ko, j * 512:(j + 1) * 512],
                                start=(ko == 4),
                                stop=(ko == 3),
                                tile_position=(0, 32 * j),
                            )
                    if has_bias:
                        nc.vector.tensor_add(psg[:], psg[:], bias_sb[:])
                    if mode == "noew":
                        h_new = ew_pool.tile([128, HP], f32, tag="h_new")
                        nc.vector.tensor_copy(h_new[:], psg[:, 0:HP])
                        nc.sync.dma_start(out[t], h_new[:])
                        if t + 1 < T:
                            for j in range(NJ):
                                tp = pt_pool.tile([128, BL], f32, tag="tp")
                                nc.tensor.matmul(
                                    tp[:], h_new[32 * j:32 * (j + 1), :],
                                    ident[32 * j:32 * (j + 1), :],
                                    is_transpose=True, tile_position=(32 * j, 0),
                                )
                                nc.vector.tensor_copy(hT[:, j, :], tp[:])
                        continue
                    acts = ew_pool.tile([128, 512], f32, tag="acts")
                    nc.scalar.activation(
                        acts[:, 0:384], psg[:, 0:384],
                        mybir.ActivationFunctionType.Sigmoid,
                    )
                    nc.scalar.activation(
                        acts[:, 384:512], psg[:, 384:512],
                        mybir.ActivationFunctionType.Tanh,
                    )
                    # free slices: i [0:128], f [128:256], o [256:384], g [384:512]
                    fc = ew_pool.tile([128, HP], f32, tag="fc")
                    nc.vector.tensor_mul(fc[:], acts[:, 128:256], c_sb[:])
                    ig = ew_pool.tile([128, HP], f32, tag="ig")
                    nc.vector.tensor_mul(ig[:], acts[:, 0:128], acts[:, 384:512])
                    nc.vector.tensor_add(c_sb[:], fc[:], ig[:])
                    tanh_c = ew_pool.tile([128, HP], f32, tag="tanh_c")
                    nc.scalar.activation(
                        tanh_c[:], c_sb[:], mybir.ActivationFunctionType.Tanh
                    )
                    h_new = ew_pool.tile([128, HP], f32, tag="h_new")
                    nc.vector.tensor_mul(h_new[:], acts[:, 256:384], tanh_c[:])
                    nc.sync.dma_start(out[t], h_new[:])
                    if mode != "notr" and t + 1 < T:
                        for j in range(NJ):
                            tp = pt_pool.tile([128, BL], f32, tag="tp")
                            nc.tensor.matmul(
                                tp[:],
                                h_new[32 * j:32 * (j + 1), :],
                                ident[32 * j:32 * (j + 1), :],
                                is_transpose=True,
                                tile_position=(32 * j, 0),
                            )
                            nc.vector.tensor_copy(hT[:, j, :], tp[:])

            if bench_loop:
                with tc.For_i(0, bench_loop, 1):
                    body()
            else:
                body()

    nc.compile()
    return nc


def _prep_inputs(feats_videos, W_embed, b_embed, W_ih, W_hh, b_ih, b_hh):
    """Host-side shard + relayout. Returns (in_maps, has_bias)."""
    f32 = np.float32
    # Combined recurrence weights: rows 0:511 = W_hh.T (h part),
    # rows 512:1023 = W_ih.T (x part).  Columns reordered to
    # col = jchunk*512 + gatepos*128 + h', gate order (i,f,o,g).
    W_cat = np.concatenate([W_hh.T, W_ih.T], axis=0).astype(f32)  # [1024, 2048]
    arr = W_cat.reshape(2 * E, 4, NJ, HP)       # [k, gate_orig, jchunk, h']
    arr = arr[:, [0, 1, 3, 2], :, :]            # gate order -> (i, f, o, g)
    wrec_np = np.ascontiguousarray(
        arr.transpose(0, 2, 1, 3).reshape(2 * E, G4)
    ).astype(np.float16)

    wembT_np = np.ascontiguousarray(W_embed.T.astype(f32))  # [F, E]

    # total gate bias, in the same [(j,b), (gatepos,h')] layout as psum
    b_g = (W_ih @ b_embed + b_ih + b_hh).astype(f32)        # [2048]
    has_bias = bool(np.any(b_g))
    bias_np = None
    if has_bias:
        bg = b_g.reshape(4, NJ, HP)[[0, 1, 3, 2], :, :]     # [gatepos, j, h']
        bias_tile = np.empty((128, 512), dtype=f32)
        for j in range(NJ):
            for gp in range(4):
                bias_tile[32 * j:32 * (j + 1), 128 * gp:128 * (gp + 1)] = bg[gp, j][None, :]
        bias_np = bias_tile

    in_maps = []
    for c in range(NC):
        fl = feats_videos[c * BL:(c + 1) * BL]              # [32, 64, 2048]
        featsT_np = np.ascontiguousarray(
            fl.transpose(2, 1, 0).reshape(F, T * BL).astype(f32)
        )
        m = {"featsT": featsT_np, "wembT": wembT_np, "wrec": wrec_np}
        if has_bias:
            m["biasg"] = bias_np
        in_maps.append(m)
    return in_maps, has_bias


def kernel(feats_videos, W_embed, b_embed, W_ih, W_hh, b_ih, b_hh):
    import sys
    if "/opt/trn_rl_repo" not in sys.path:
        sys.path.insert(0, "/opt/trn_rl_repo")
    from concourse.bass_utils import run_bass_kernel_spmd

    feats_videos = np.asarray(feats_videos, dtype=np.float32)
    W_embed = np.asarray(W_embed, dtype=np.float32)
    b_embed = np.asarray(b_embed, dtype=np.float32)
    W_ih = np.asarray(W_ih, dtype=np.float32)
    W_hh = np.asarray(W_hh, dtype=np.float32)
    b_ih = np.asarray(b_ih, dtype=np.float32)
    b_hh = np.asarray(b_hh, dtype=np.float32)

    in_maps, has_bias = _prep_inputs(
        feats_videos, W_embed, b_embed, W_ih, W_hh, b_ih, b_hh
    )
    if has_bias not in _prog_cache:
        _prog_cache[has_bias] = _build_program(has_bias)
    nc = _prog_cache[has_bias]

    res = run_bass_kernel_spmd(nc, in_maps, list(range(NC)))
    outs = []
    for c in range(NC):
        r = res.results[c]["out"]                            # [64, 128, 128]
        o = r.reshape(T, NJ, BL, HP).transpose(2, 0, 1, 3).reshape(BL, T, H)
        outs.append(o)
    return np.concatenate(outs, axis=0).astype(np.float32)   # [256, 64, 512]


# revision 12
# speedup vs baseline: 675.5600x; 346.7675x over previous
"""LSTM encoder kernel for TRN2 (8 NeuronCores, data-parallel over batch).

Reference computation:
  x = feats @ W_embed.T + b_embed            [B,T,F] -> [B,T,E]
  per t: gates = x_t @ W_ih.T + b_ih + h @ W_hh.T + b_hh
         i,f,g,o = split(gates); c = sig(f)*c + sig(i)*tanh(g); h = sig(o)*tanh(c)
  out = stacked h                            [B,T,H]

B=256, T=64, F=2048, E=512, H=512.  Shard B over 8 cores (B_L=32 each).

Device-side design (per core):
  Phase 1: xT[E, T*B_L] = W_emb @ featsT in float32r (feats pre-transposed on
           host so the per-step slice xT[:, t*32:(t+1)*32] is directly the
           lhsT of step t).
  Phase 2: per step, col-tiled fp16 matmuls (tile_position=(0,32j)) produce
           gates in PSUM with layout [(j=h-chunk, b) partitions, (gate,h')
           free] so all elementwise LSTM math runs on full 128 partitions.
           Gate columns are pre-permuted on host to (i,f,o,g) so one sigmoid
           covers free[0:384] and one tanh covers free[384:512].
           h is re-transposed for the next step with 4 row-tiled PE
           transposes.
  Matmuls: embedding fp32r (1 cyc/row at N>=256), recurrence fp16 (1 cyc/row,
  col-tiling legal, ~1.4e-3 rel err end to end).  Elementwise fp32.
"""

import numpy as np

B, T, F, E, H = 256, 64, 2048, 512, 512
NC = 8
BL = B // NC          # 32 batch rows per core
G4 = 4 * H            # 2048 gate columns
NJ = 4                # h-chunks of 128 (and batch col-groups of 32)
HP = H // NJ          # 128

_prog_cache = {}


def _build_program(has_bias: bool, bench_loop: int = 0, mode: str = "all"):
    import concourse.bass as bass
    import concourse.tile as tile
    from concourse import bacc, mybir
    from concourse.masks import make_identity
    from contextlib import ExitStack

    f32 = mybir.dt.float32
    f32r = mybir.dt.float32r
    f16 = mybir.dt.float16

    nc = bacc.Bacc("TRN2", target_bir_lowering=False, debug=False)

    featsT = nc.dram_tensor("featsT", [F, T * BL], f16, kind="ExternalInput").ap()
    wembT = nc.dram_tensor("wembT", [F, E], f16, kind="ExternalInput").ap()
    wrec = nc.dram_tensor("wrec", [2 * E, G4], f16, kind="ExternalInput").ap()
    if has_bias:
        biasg = nc.dram_tensor("biasg", [128, 512], f32, kind="ExternalInput").ap()
    out = nc.dram_tensor("out", [T, 128, HP], f32, kind="ExternalOutput").ap()

    with tile.TileContext(nc) as tc:
        with ExitStack() as ctx:
            const_pool = ctx.enter_context(tc.tile_pool(name="const", bufs=1))
            state_pool = ctx.enter_context(tc.tile_pool(name="state", bufs=1))
            embin_pool = ctx.enter_context(tc.tile_pool(name="emb_in", bufs=2))
            pg_pool = ctx.enter_context(
                tc.tile_pool(name="pg", bufs=4, space="PSUM"))
            embps_pool = pg_pool
            pt_pool = ctx.enter_context(
                tc.tile_pool(name="pt", bufs=4, space="PSUM"))
            ew_pool = ctx.enter_context(tc.tile_pool(name="ew", bufs=3))

            # I32 replicated per 32-partition block (transpose rhs must share
            # the lhsT base partition).
            ident = const_pool.tile([128, 32], f32)
            for j in range(NJ):
                make_identity(nc, ident[32 * j:32 * (j + 1), :])

            xT = const_pool.tile([128, NJ, T * BL], f16)         # 2 MB
            wrec_sb = const_pool.tile([128, 8, G4], f16)         # 4 MB
            nc.sync.dma_start(
                wrec_sb[:], wrec.rearrange("(ko p) n -> p ko n", p=128)
            )
            wemb_sb = const_pool.tile([128, 16, E], f16)         # 2 MB
            nc.sync.dma_start(
                wemb_sb[:], wembT.rearrange("(ko p) m -> p ko m", p=128)
            )
            if has_bias:
                bias_sb = const_pool.tile([128, 512], f32)
                nc.sync.dma_start(bias_sb[:], biasg[:])

            # h transposed: hT[h', j*32+b]; lhsT of K-subtile ko is
            # hT[:, 32*ko:32*(ko+1)]
            hT = state_pool.tile([128, NJ * BL], f16)
            c_sb = state_pool.tile([128, HP], f32)               # cell state

            def body():
                nc.vector.memzero(hT[:])
                nc.vector.memzero(c_sb[:])

                if mode == "rec":
                    pass  # skip embedding (xT left stale)
                # ---- phase 1: xT = W_emb @ featsT (fp32r) ----
                NCH = 256
                NCHUNKS = 0 if mode == "rec" else (T * BL) // NCH
                featsT_r = featsT.rearrange("(ko p) n -> p ko n", p=128)
                for nch in range(NCHUNKS):
                    rhs = embin_pool.tile([128, 16, NCH], f16, tag="embrhs")
                    nc.sync.dma_start(
                        rhs[:], featsT_r[:, :, nch * NCH:(nch + 1) * NCH]
                    )
                    for m in range(NJ):
                        ps = embps_pool.tile([128, 512], f32, tag="psg", name=f"embps_{nch}_{m}")[:, :NCH]
                        for ko in range(16):
                            nc.tensor.matmul(
                                ps[:],
                                wemb_sb[:, ko, m * 128:(m + 1) * 128],
                                rhs[:, ko, :],
                                start=(ko == 0),
                                stop=(ko == 15),
                            )
                        nc.vector.tensor_copy(
                            xT[:, m, nch * NCH:(nch + 1) * NCH], ps[:]
                        )

                # ---- phase 2: recurrence (fp16 col-tiled matmuls) ----
                # x-part MMs are emitted XAHEAD steps early so the PE has
                # independent work during step t's elementwise chain.
                XAHEAD = 3
                NT = 0 if mode == "emb" else T
                psgs = {}

                def emit_x(t):
                    psgs[t] = pg_pool.tile([128, 512], f32, tag="psg", name=f"psg_{t}")
                    for ko in range(4):
                        lhsT = xT[:, ko, t * BL:(t + 1) * BL]
                        for j in range(NJ):
                            nc.tensor.matmul(
                                psgs[t][32 * j:32 * (j + 1), :],
                                lhsT,
                                wrec_sb[:, ko + 4, j * 512:(j + 1) * 512],
                                start=(ko == 0), stop=False,
                                tile_position=(0, 32 * j),
                                skip_group_check=True,
                            )

                for t in range(min(XAHEAD, NT)):
                    emit_x(t)
                for t in range(NT):
                    psg = psgs.pop(t)
                    for ko in range(4):
                        lhsT = hT[:, 32 * ko:32 * (ko + 1)]
                        for j in range(NJ):
                            nc.tensor.matmul(
                                psg[32 * j:32 * (j + 1), :],
                                lhsT,
                                wrec_sb[:, ko, j * 512:(j + 1) * 512],
                                start=False, stop=(ko == 3),
                                tile_position=(0, 32 * j),
                                skip_group_check=True,
                            )
                    if t + XAHEAD < NT:
                        emit_x(t + XAHEAD)
                    if has_bias:
                        nc.vector.tensor_add(psg[:], psg[:], bias_sb[:])
                    if mode == "noew":
                        h_new = ew_pool.tile([128, HP], f32, tag="h_new")
                        nc.vector.tensor_copy(h_new[:], psg[:, 0:HP])
                    else:
                        # gate order (f,i,g,o): f [0:128], i [128:256],
                        # g [256:384], o [384:512]
                        acts = ew_pool.tile([128, 512], f32, tag="acts")
                        nc.scalar.activation(
                            acts[:, 0:256], psg[:, 0:256],
                            mybir.ActivationFunctionType.Sigmoid,
                        )
                        nc.scalar.activation(
                            acts[:, 256:384], psg[:, 256:384],
                            mybir.ActivationFunctionType.Tanh,
                        )
                        nc.scalar.activation(
                            acts[:, 384:512], psg[:, 384:512],
                            mybir.ActivationFunctionType.Sigmoid,
                        )
                        fc = ew_pool.tile([128, HP], f32, tag="fc")
                        nc.vector.tensor_mul(fc[:], acts[:, 0:128], c_sb[:])
                        ig = ew_pool.tile([128, HP], f32, tag="ig")
                        nc.vector.tensor_mul(ig[:], acts[:, 128:256],
                                             acts[:, 256:384])
                        nc.vector.tensor_add(c_sb[:], fc[:], ig[:])
                        tanh_c = ew_pool.tile([128, HP], f32, tag="tanh_c")
                        nc.scalar.activation(
                            tanh_c[:], c_sb[:],
                            mybir.ActivationFunctionType.Tanh,
                        )
                        h_new = ew_pool.tile([128, HP], f32, tag="h_new")
                        nc.vector.tensor_mul(h_new[:], acts[:, 384:512],
                                             tanh_c[:])
                    nc.sync.dma_start(out[t], h_new[:])
                    if mode != "notr" and t + 1 < NT:
                        for j in range(NJ):
                            tp = pt_pool.tile([128, BL], f32, tag="tp")
                            nc.tensor.matmul(
                                tp[:],
                                h_new[32 * j:32 * (j + 1), :],
                                ident[32 * j:32 * (j + 1), :],
                                is_transpose=True,
                                tile_position=(32 * j, 0),
                            )
                            nc.vector.tensor_copy(
                                hT[:, 32 * j:32 * (j + 1)], tp[:])

            if bench_loop:
                with tc.For_i(0, bench_loop, 1):
                    body()
            else:
                body()

    nc.compile()
    return nc


def _prep_inputs(feats_videos, W_embed, b_embed, W_ih, W_hh, b_ih, b_hh):
    """Host-side shard + relayout. Returns (in_maps, has_bias)."""
    f32 = np.float32
    # Combined recurrence weights: rows 0:511 = W_hh.T (h part),
    # rows 512:1023 = W_ih.T (x part).  Columns reordered to
    # col = jchunk*512 + gatepos*128 + h', gate order (i,f,o,g).
    W_cat = np.concatenate([W_hh.T, W_ih.T], axis=0).astype(f32)  # [1024, 2048]
    arr = W_cat.reshape(2 * E, 4, NJ, HP)       # [k, gate_orig, jchunk, h']
    arr = arr[:, [1, 0, 2, 3], :, :]            # gate order -> (f, i, g, o)
    wrec_np = np.ascontiguousarray(
        arr.transpose(0, 2, 1, 3).reshape(2 * E, G4)
    ).astype(np.float16)

    wembT_np = np.ascontiguousarray(W_embed.T).astype(np.float16)  # [F, E]

    # total gate bias, in the same [(j,b), (gatepos,h')] layout as psum
    b_g = (W_ih @ b_embed + b_ih + b_hh).astype(f32)        # [2048]
    has_bias = bool(np.any(b_g))
    bias_np = None
    if has_bias:
        bg = b_g.reshape(4, NJ, HP)[[1, 0, 2, 3], :, :]     # [gatepos, j, h']
        bias_tile = np.empty((128, 512), dtype=f32)
        for j in range(NJ):
            for gp in range(4):
                bias_tile[32 * j:32 * (j + 1), 128 * gp:128 * (gp + 1)] = bg[gp, j][None, :]
        bias_np = bias_tile

    in_maps = []
    for c in range(NC):
        fl = feats_videos[c * BL:(c + 1) * BL]              # [32, 64, 2048]
        featsT_np = np.ascontiguousarray(
            fl.transpose(2, 1, 0).reshape(F, T * BL)
        ).astype(np.float16)
        m = {"featsT": featsT_np, "wembT": wembT_np, "wrec": wrec_np}
        if has_bias:
            m["biasg"] = bias_np
        in_maps.append(m)
    return in_maps, has_bias


def kernel(feats_videos, W_embed, b_embed, W_ih, W_hh, b_ih, b_hh):
    import sys
    if "/opt/trn_rl_repo" not in sys.path:
        sys.path.insert(0, "/opt/trn_rl_repo")
    from concourse.bass_utils import run_bass_kernel_spmd

    feats_videos = np.asarray(feats_videos, dtype=np.float32)
    W_embed = np.asarray(W_embed, dtype=np.float32)
    b_embed = np.asarray(b_embed, dtype=np.float32)
    W_ih = np.asarray(W_ih, dtype=np.float32)
    W_hh = np.asarray(W_hh, dtype=np.float32)
    b_ih = np.asarray(b_ih, dtype=np.float32)
    b_hh = np.asarray(b_hh, dtype=np.float32)

    in_maps, has_bias = _prep_inputs(
        feats_videos, W_embed, b_embed, W_ih, W_hh, b_ih, b_hh
    )
    if has_bias not in _prog_cache:
        _prog_cache[has_bias] = _build_program(has_bias)
    nc = _prog_cache[has_bias]

    res = run_bass_kernel_spmd(nc, in_maps, list(range(NC)))
    outs = []
    for c in range(NC):
        r = res.results[c]["out"]                            # [64, 128, 128]
        o = r.reshape(T, NJ, BL, HP).transpose(2, 0, 1, 3).reshape(BL, T, H)
        outs.append(o)
    return np.concatenate(outs, axis=0).astype(np.float32)   # [256, 64, 512]
